# revision 41
# baseline (speedup 1.0000x reference)
"""Trainium2 Bass kernel for a binarized-weight MLP (BNN MNIST-style):

    h   = x @ sign(W1).T + b1      # fc1, binarized weights
    h   = clip(h, -1, 1)           # Hardtanh
    out = h @ W2.T + b2            # fc2

Shapes: x [8192, 784] f32, W1 [4096, 784], b1 [4096], W2 [10, 4096], b2 [10].

Strategy (data-parallel over 8 NeuronCores):
  - Shard batch 8192 -> 1024 rows/core; replicate weights. All matmuls bf16
    moving (sign(W1) exact in fp8 stationary), fp32 PSUM.  (fp8 DoubleRow
    was measured 2x per contraction but requires BOTH operands fp8; x needs
    ~7 significant bits, and an fp8 hi/lo split doubles the contraction,
    exactly cancelling the gain — so bf16 rate is the PE floor here.)
  - fc1 contraction K = 784 x-rows + 2 bias rows (b1 hi/lo fp8 split with
    ones-columns in x) = 786 rows = 6 full 128-row k-tiles + an 18-row tail.
  - The 18-row tail is ROW-TILED: hidden tiles are processed in quads of 4,
    and one PE slot runs 4 concurrent K=18 matmuls at tile_position
    (0/32/64/96, 0) - one per ht in the quad - so the tail costs 1 slot per
    quad instead of 4.  Slot order per quad: packed tail first (start=True
    for all 4 PSUM groups), then ht-by-ht k0..5 with stop at k5, so each
    ht's Hardtanh (DVE tensor_scalar min/max) pipelines inside the quad.
  - fc2 (M=10) is COL-TILED: 4 concurrent matmuls at tile_position
    (0, 0/32/64/96) accumulate W2-slices @ h into partition slices
    32r..32r+9.  fc2 is emitted in LARGE BLOCKS (nt0's block after nt1-q0;
    nt1's q0..5 after nt1-q6, q6 mid-q7, q7 at the end) instead of
    per-quad bursts: each switch between full-array / row-tiled /
    col-tiled matmul modes costs ~100-320ns of PE stall.  fc2 matmuls are
    N=256 column halves, each half accumulating into its OWN PSUM bank
    (tags ps2_0/ps2_1): the tile framework serializes ops touching one
    tile, so per-half tiles let the two end chains overlap.
  - End chain per half: ps2 -> s2 copy (ACT owns half 0, DVE half 1, into
    separate s2 half tiles) -> selector matmul (sel[32r+o, o] = 1, b2 on
    sel row 127 vs the ones-rows of s2; the two halves run CONCURRENTLY
    via col-tiling into psel rows 0:10 / 32:42) -> out-copy -> out-DMA
    (half 0 on sync queue, half 1 on scalar).  Output is bf16 (adds
    ~0.02% error, halves the final DMA).  The last k-chain (ht31) and its
    Hardtanh run in column halves so the end chain starts half a slot
    early.
  - 26 PE warm-up matmuls (N=256, alternating between the two ps2-bank
    scratch tiles so they carry no WAW chain and pipeline at stream rate)
    bridge the ~11.5us DMA head with zero PE idle: any idle gap resets the
    HAM clock-ramp window and leaves the first quads at 1.2 instead of
    2.4 GHz.
  - quad 0 of nt0 runs kt-outermost (all 4 ht per arriving x k-tile,
    ~864ns of work per tile, matching the ~0.9us/tile DMA rate); w1 pairs
    0/1 are DMA'd in half-pair slices so the first k-tiles unlock early.
  - Per-core output is out^T [10, 1024] bf16; host gathers + transposes.
"""

import numpy as np
import ml_dtypes
from contextlib import ExitStack

import concourse.bass as bass
import concourse.mybir as mybir
import concourse.tile as tile
from concourse import bacc
from concourse import bass_utils

BF16_NP = ml_dtypes.bfloat16
FP8_NP = ml_dtypes.float8_e4m3
BF16 = mybir.dt.bfloat16
FP8 = mybir.dt.float8e4
F32 = mybir.dt.float32

BATCH, IN, HID, OUT = 8192, 784, 4096, 10
NCORES = 8
B_CORE = BATCH // NCORES        # 1024
NT = B_CORE // 512              # 2 batch n-tiles of 512 per core
HT = HID // 128                 # 32 hidden tiles
NQ = HT // 4                    # 8 quads of 4 hidden tiles
KF = 6                          # full 128-row k-tiles (768 rows)
KTAIL = IN + 2 - KF * 128       # 18 tail rows (16 x-rows + 2 bias rows)
N_WARMUP = 26                   # PE warm-up matmuls (HAM un-throttle)

_CACHE = {}


def _build():
    """Build + compile the Bacc graph once per process."""
    if "nc" in _CACHE:
        return _CACHE["nc"]

    nc = bacc.Bacc("TRN2", target_bir_lowering=False, debug=False,
                   num_devices=NCORES)
    xt_d = nc.dram_tensor("xt", [NT, 128, KF * 512], BF16,
                          kind="ExternalInput").ap()
    xt6_d = nc.dram_tensor("xt6", [NT, 128, 512], BF16,
                           kind="ExternalInput").ap()
    # sign(W1) in {-1,0,+1} is exact in fp8 (the moving tensor stays bf16,
    # which sets the matmul rate) — halves the dominant DMA stream.
    w1_d = nc.dram_tensor("w1", [HT // 2, 128, 2 * KF * 128], FP8,
                          kind="ExternalInput").ap()
    w1k7_d = nc.dram_tensor("w1k7", [128, NQ * 128], FP8,
                            kind="ExternalInput").ap()
    w2_d = nc.dram_tensor("w2", [128, HT * OUT], BF16,
                          kind="ExternalInput").ap()
    sel_d = nc.dram_tensor("sel", [128, OUT], BF16,
                           kind="ExternalInput").ap()
    out_d = nc.dram_tensor("out", [OUT, B_CORE], BF16,
                           kind="ExternalOutput").ap()

    # Raw SBUF tensor for PE warm-up matmuls: contents irrelevant, results
    # discarded into a scratch PSUM bank that is later reused by the
    # selector matmul.
    warm_sb = nc.alloc_sbuf_tensor("warm_raw", [128, 512], BF16).ap()

    with tile.TileContext(nc) as tc:
        with ExitStack() as ctx:
            wpool = ctx.enter_context(tc.tile_pool(name="w1", bufs=1))
            xpool = ctx.enter_context(tc.tile_pool(name="x", bufs=1))
            cpool = ctx.enter_context(tc.tile_pool(name="const", bufs=1))
            # h: single rotating tag; fc2 runs as large deferred blocks, so
            # all 32 of an nt's h tiles (plus the next nt's first quads)
            # must stay live -> 40 bufs.
            hpool = ctx.enter_context(tc.tile_pool(name="h", bufs=40))
            s2pool = ctx.enter_context(tc.tile_pool(name="s2", bufs=1))
            # PSUM bank budget (8 banks of [128,512]f32):
            #   ps1 x6 (rotating; 6 > 4/quad + 1 so a quad's packed matmuls
            #   never WAR-wait on the previous quad's in-flight Hardtanh
            #   read of the same bank) + 2 banks (tags ps2_0/ps2_1) shared
            #   sequentially by warmups / ps2(nt) / psel(nt): the fc2 /
            #   selector chain is split into column halves, each half's
            #   chain living in its OWN bank so the two chains overlap.
            ps1pool = ctx.enter_context(
                tc.tile_pool(name="ps1", bufs=6, space="PSUM"))
            ps2pool = ctx.enter_context(
                tc.tile_pool(name="ps2", bufs=1, space="PSUM"))

            w1k7_sb = cpool.tile([128, NQ * 128], FP8, tag="w1k7")
            w2_sb = cpool.tile([128, HT * OUT], BF16, tag="w2")
            sel_sb = cpool.tile([128, OUT], BF16, tag="sel")
            xt6_sb = [cpool.tile([128, 512], BF16, tag=f"xt6_{nt}",
                                 name=f"xt6_{nt}")
                      for nt in range(NT)]

            # PE warm-up: HAM keeps the PE at 1.2 GHz until ~3.4us of
            # sustained matmul activity, and an idle gap resets the window;
            # run dummy matmuls back-to-back until the first input tiles
            # have landed (~11.5us).  They write 4 rotating column slices
            # of one scratch PSUM tile so they carry no WAW chain: each is
            # always ready, pipelines at stream rate, and the scheduler's
            # priority heap keeps all of them ahead of the (not-yet-ready)
            # real matmuls.
            pswarm = [ps2pool.tile([128, 256], F32, tag=f"ps2_{h}",
                                   name=f"pswarm_{h}") for h in range(2)]
            for i in range(N_WARMUP):
                nc.tensor.matmul(pswarm[i % 2][:], warm_sb[:, 0:128],
                                 warm_sb[:, 0:256], start=True, stop=True,
                                 skip_group_check=True)

            # Input DMAs on three parallel HWDGE queues, ordered so the
            # first-consumed tiles land first:
            #   sync:   w1k7, w1[0..1] per-kt slices, w1[2..15] pairs
            #   gpsimd: xt6[0], x(0, 0/2/4)
            #   scalar: x(0, 1/3/5), w2, sel, xt6[1], x(1, k*)
            # w1 in 2-ht pairs (halves DMA-issue count); consumers slice.
            w1_p = [wpool.tile([128, 2 * KF * 128], FP8, tag=f"w1_{hp}",
                               name=f"w1_{hp}")
                    for hp in range(HT // 2)]

            def w1_slice(ht, kt):
                base = kt * 256 + (ht % 2) * 128
                return w1_p[ht // 2][:, base:base + 128]

            # x as one wide tile per nt; per-kt column-slice DMAs so the
            # consumers' arrival pacing is fine-grained.
            x_w = [xpool.tile([128, KF * 512], BF16, tag=f"x_{nt}",
                              name=f"x_{nt}")
                   for nt in range(NT)]

            def x_slice(nt, kt):
                return x_w[nt][:, kt * 512:(kt + 1) * 512]

            def x_dma(eng, nt, kt):
                eng.dma_start(x_slice(nt, kt), xt_d[nt, :,
                                                    kt * 512:(kt + 1) * 512])

            nc.sync.dma_start(w1k7_sb[:], w1k7_d)
            # pairs 0/1 in half-pair slices (kt 0-2, then kt 3-5; 768-byte
            # partition lines) so quad-0's kt-outermost matmuls for the
            # first three k-tiles unlock as one early chunk
            for half in range(2):
                for hp in (0, 1):
                    lo, hi = half * 768, (half + 1) * 768
                    nc.sync.dma_start(w1_p[hp][:, lo:hi],
                                      w1_d[hp, :, lo:hi])
            for hp in range(2, HT // 2):
                nc.sync.dma_start(w1_p[hp][:], w1_d[hp])

            nc.gpsimd.dma_start(xt6_sb[0][:], xt6_d[0])
            for kt in (0, 2, 4):
                x_dma(nc.gpsimd, 0, kt)

            for kt in (1, 3, 5):
                x_dma(nc.scalar, 0, kt)
            nc.scalar.dma_start(w2_sb[:], w2_d)
            nc.scalar.dma_start(sel_sb[:], sel_d)
            nc.scalar.dma_start(xt6_sb[1][:], xt6_d[1])
            for kt in range(KF):
                x_dma(nc.scalar, 1, kt)

            # deferred actions, popped between quads so PE-queue stalls on
            # not-yet-ready DVE/ACT results are avoided
            deferred = []

            def sel_chain(nt):
                # ps2 partition groups {0,32,64,96}+0..9 -> out via one
                # selector matmul per column half; b2 rides on sel partition
                # 127 against the ones rows of s2 (pre-memset to 1.0).  Each
                # half has its OWN ps2/s2/psel/out tiles and ONE owning
                # engine (ACT: half 0, DVE: half 1); for nt1 the h1 tiles
                # even sit in a different PSUM bank, so the two
                # copy -> selector mm -> out-copy -> out-DMA chains overlap
                # fully at the end of the kernel.  The two selector matmuls
                # run concurrently via column tiling: h0 -> psel rows 0:10
                # at (0,0), h1 -> psel rows 32:42 at (0,32).
                pselh = [ps2pool.tile([128, 256], F32, tag=f"ps2_{h}",
                                      name=f"psel_{nt}_{h}")
                         for h in range(2)]
                rows = [(0, OUT), (32, 32 + OUT)]

                def copy(h):
                    def fn():
                        ps2t = ps2_t[nt][h]
                        if h == 0:
                            nc.scalar.activation(
                                s2h[nt][0][0:106, :], ps2t[0:106, 0:256],
                                mybir.ActivationFunctionType.Identity)
                        else:
                            nc.vector.tensor_scalar(
                                s2h[nt][1][0:106, :], ps2t[0:106, 0:256],
                                1.0, None, op0=mybir.AluOpType.mult)
                    return fn

                def mm(h):
                    def fn():
                        r0, r1 = rows[h]
                        nc.tensor.matmul(pselh[h][r0:r1, 0:256], sel_sb[:],
                                         s2h[nt][h][:],
                                         start=True, stop=True,
                                         tile_position=(0, r0),
                                         skip_group_check=True)
                    return fn

                def out(h):
                    def fn():
                        lo = nt * 512 + h * 256
                        r0, r1 = rows[h]
                        if h == 0:
                            nc.scalar.activation(
                                outh[nt][0][:], pselh[0][r0:r1, 0:256],
                                mybir.ActivationFunctionType.Identity)
                            nc.sync.dma_start(out_d[:, lo:lo + 256],
                                              outh[nt][0][:])
                        else:
                            nc.vector.tensor_scalar(
                                outh[nt][1][:], pselh[1][r0:r1, 0:256],
                                1.0, None, op0=mybir.AluOpType.mult)
                            nc.scalar.dma_start(out_d[:, lo:lo + 256],
                                                outh[nt][1][:])
                    return fn

                return [copy(0), copy(1), mm(0), mm(1), out(0), out(1)]

            # ps2/psel tiles rotate through the single shared PSUM bank in
            # strict sequence: pswarm -> ps2_0 -> psel_0 -> ps2_1 -> psel_1.
            ps2_t = [None, None]
            s2h = [[s2pool.tile([128, 256], BF16, tag=f"s2_{nt}_{h}",
                                name=f"s2_{nt}_{h}")
                    for h in range(2)] for nt in range(NT)]
            outh = [[s2pool.tile([OUT, 256], BF16, tag=f"out_{nt}_{h}",
                                 name=f"out_{nt}_{h}")
                     for h in range(2)] for nt in range(NT)]
            # fc2 only ever writes partition groups 32r..32r+9 of ps2, and
            # the ACT copy in sel_chain reads ps2[0:106) into s2 whose rows
            # 106..127 must be 1.0 (bias row) — stale PSUM/SBUF could hold
            # NaN and NaN*0 = NaN in the selector matmul.  Full-tile
            # memsets (start=True matmuls overwrite their elements
            # regardless).
            for nt in range(NT):
                for h in range(2):
                    nc.gpsimd.memset(s2h[nt][h][:], 1.0)

            def make_ps2(nt):
                ps2_t[nt] = [ps2pool.tile([128, 256], F32, tag=f"ps2_{h}",
                                          name=f"ps2_{nt}_{h}")
                             for h in range(2)]
                for h in range(2):
                    nc.vector.memset(ps2_t[nt][h][:], 0.0)

            # fc2 col-tiled matmuls, deferred into large blocks.  hs_all
            # keeps every quad's h tiles; flush_fc2 emits one block of
            # col-tiled matmuls (q-major: 4 concurrent col groups, quads
            # serialize within a group).
            hs_all = {}                      # (nt, q) -> [h tiles r=0..3]

            def flush_fc2(nt, q_lo, q_hi):
                for q in range(q_lo, q_hi):
                    hs = hs_all.pop((nt, q))
                    for r in range(4):
                        w2s = w2_sb[:, (4 * q + r) * OUT:
                                    (4 * q + r + 1) * OUT]
                        for h in range(2):
                            lo, hi = h * 256, (h + 1) * 256
                            nc.tensor.matmul(
                                ps2_t[nt][h][32 * r:32 * r + OUT, 0:256],
                                w2s, hs[r][:, lo:hi],
                                start=(q == 0), stop=(q == NQ - 1),
                                tile_position=(0, 32 * r),
                                skip_group_check=True)

            for nt in range(NT):
                if nt == 0:
                    make_ps2(0)
                for q in range(NQ):
                    ps1 = [ps1pool.tile([128, 512], F32, tag="ps1",
                                        name=f"ps1_{nt}_{q}_{r}")
                           for r in range(4)]

                    def tail_mms(start, stop):
                        # packed 18-row tail: 4 concurrent row-tiled matmuls
                        for r in range(4):
                            nc.tensor.matmul(
                                ps1[r][:],
                                w1k7_sb[32 * r:32 * r + KTAIL,
                                        q * 128:(q + 1) * 128],
                                xt6_sb[nt][32 * r:32 * r + KTAIL, :],
                                start=start, stop=stop,
                                tile_position=(32 * r, 0),
                                skip_group_check=True)

                    tail_mms(True, False)
                    hs = []
                    if nt == 0 and q == 0:
                        # kt-outermost for the very first quad: each
                        # arriving x k-tile unlocks 4 matmuls (~864ns of PE
                        # work), matching the ~0.9us/tile DMA arrival rate,
                        # so the DMA-paced head has few PE stalls.
                        for kt in range(KF):
                            for r in range(4):
                                nc.tensor.matmul(
                                    ps1[r][:], w1_slice(r, kt),
                                    x_slice(0, kt),
                                    start=False, stop=(kt == KF - 1),
                                    skip_group_check=True)
                        for r in range(4):
                            h = hpool.tile([128, 512], BF16, tag="h",
                                           name=f"h_0_{r}")
                            nc.vector.tensor_scalar(
                                h[:], ps1[r][:], 1.0, -1.0,
                                op0=mybir.AluOpType.min,
                                op1=mybir.AluOpType.max)
                            hs.append(h)
                    else:
                        for r in range(4):
                            ht = 4 * q + r
                            if nt == 1 and q == NQ - 1 and r == 3:
                                # the very last k-chain runs in column
                                # halves so its Hardtanh (and with it the
                                # whole end chain) starts half a slot
                                # earlier
                                for kt in range(KF):
                                    for hh in range(2):
                                        lo = kt * 512 + hh * 256
                                        nc.tensor.matmul(
                                            ps1[r][:, hh * 256:
                                                 (hh + 1) * 256],
                                            w1_slice(ht, kt),
                                            x_w[nt][:, lo:lo + 256],
                                            start=False,
                                            stop=(kt == KF - 1),
                                            skip_group_check=True)
                            else:
                                for kt in range(KF):
                                    nc.tensor.matmul(
                                        ps1[r][:],
                                        w1_slice(ht, kt),
                                        x_slice(nt, kt),
                                        start=False, stop=(kt == KF - 1),
                                        skip_group_check=True)
                            if nt == 1 and q == NQ - 1 and r == 1:
                                # q6's h tiles are long done: retire its
                                # fc2 matmuls here so only q7's remain
                                # after the last k-chain
                                flush_fc2(1, 6, 7)
                            h = hpool.tile([128, 512], BF16, tag="h",
                                           name=f"h_{nt}_{ht}")
                            # Hardtanh + downcast: h = max(min(ps1, 1), -1)
                            if nt == 1 and q == NQ - 1 and r == 3:
                                # last hidden tile: column halves, so the
                                # end chain can start after half the op
                                for hh in range(2):
                                    lo, hi = hh * 256, (hh + 1) * 256
                                    nc.vector.tensor_scalar(
                                        h[:, lo:hi], ps1[r][:, lo:hi],
                                        1.0, -1.0,
                                        op0=mybir.AluOpType.min,
                                        op1=mybir.AluOpType.max)
                            else:
                                nc.vector.tensor_scalar(
                                    h[:], ps1[r][:], 1.0, -1.0,
                                    op0=mybir.AluOpType.min,
                                    op1=mybir.AluOpType.max)
                            hs.append(h)
                    hs_all[(nt, q)] = hs
                    # fc2 block placements + deferred sel-chain pops: nt0's
                    # fc2 runs right after nt1-q0's fc1 (all nt0 Hardtanhs
                    # done); its sel chain pops over the following quads.
                    # nt1's fc2 q0..5 runs after nt1-q6 (WAR on ps2 vs
                    # nt0's copy is long settled), q6..7 at the end.
                    if nt == 1 and q == 0:
                        flush_fc2(0, 0, NQ)
                    elif nt == 1 and q == 6:
                        flush_fc2(1, 0, 6)
                    if nt == 1 and deferred:
                        deferred.pop(0)()
                        if deferred:
                            deferred.pop(0)()
                    if nt == 1 and q == 5:
                        # after psel_0's out-copies are emitted (pops at
                        # q<=2) — the shared-bank WAW chain must see them
                        # before ps2_1's memset; late enough that the h1
                        # tile borrowed from the ps1 ring only adds ring
                        # pressure for the last two quads
                        make_ps2(1)
                deferred.extend(sel_chain(nt))
            flush_fc2(1, 7, NQ)
            for fn in deferred:
                fn()

    nc.compile()
    _CACHE["nc"] = nc
    return nc


def _prep_inputs(x, W1, b1, W2, b2):
    """Host-side shard + layout prep. Returns in_maps for the 8 cores."""
    x = np.asarray(x, dtype=np.float32)
    W1 = np.asarray(W1, dtype=np.float32)
    b1 = np.asarray(b1, dtype=np.float32)
    W2 = np.asarray(W2, dtype=np.float32)
    b2 = np.asarray(b2, dtype=np.float32)

    K = IN + 2                                           # 786
    # fc1 weight in fp8 (sign values exact), augmented with two bias rows
    # (hi + lo fp8 split of b1; residual ~0.4% of b1, negligible vs the
    # Hardtanh clip scale).
    w1aug = np.zeros((K, HID), dtype=np.float32)
    w1aug[:IN] = np.sign(W1).T
    b1_hi = b1.astype(FP8_NP).astype(np.float32)
    w1aug[IN] = b1_hi
    w1aug[IN + 1] = b1 - b1_hi
    w1aug = w1aug.astype(FP8_NP)

    # full k-tiles in 2-ht pairs: [hp, p, kt*256 + j*128 + m]
    #   = w1aug[kt*128+p, (2hp+j)*128+m]
    w1_host = np.ascontiguousarray(
        w1aug[:KF * 128].reshape(KF, 128, HT // 2, 2 * 128)
        .transpose(2, 1, 0, 3).reshape(HT // 2, 128, 2 * KF * 128))

    # 18-row tail, packed for 4-way row tiling:
    # [32r+j, q*128+m] = w1aug[768+j, (4q+r)*128+m]
    w1k7 = np.zeros((128, NQ * 128), dtype=FP8_NP)
    tail = w1aug[KF * 128:].reshape(KTAIL, NQ, 4, 128)   # [j, q, r, m]
    for r in range(4):
        w1k7[32 * r:32 * r + KTAIL] = tail[:, :, r, :].reshape(KTAIL, -1)

    # fc2 weight: [p, ht*10+o] = W2[o, ht*128+p]
    w2_host = np.ascontiguousarray(
        W2.T.astype(BF16_NP).reshape(HT, 128, OUT)
        .transpose(1, 0, 2).reshape(128, HT * OUT))

    # selector for the 4-way fc2 partition-group reduction, b2 on row 127
    sel_host = np.zeros((128, OUT), dtype=BF16_NP)
    for r in range(4):
        for o in range(OUT):
            sel_host[32 * r + o, o] = 1
    sel_host[127, :] = b2.astype(BF16_NP)

    # x augmented with ones-columns matching the two b1 rows.
    x_aug = np.zeros((BATCH, K), dtype=BF16_NP)
    x_aug[:, :IN] = x.astype(BF16_NP)
    x_aug[:, IN] = 1
    x_aug[:, IN + 1] = 1

    in_maps = []
    for c in range(NCORES):
        xc = x_aug[c * B_CORE:(c + 1) * B_CORE]          # [1024, 786]
        # full k-tiles: [nt, p, kt*512+b] = xc[nt*512+b, kt*128+p]
        xt = np.ascontiguousarray(
            xc[:, :KF * 128].reshape(NT, 512, KF, 128)
            .transpose(0, 3, 2, 1).reshape(NT, 128, KF * 512))
        # tail block replicated at partition bases 0/32/64/96
        xt6 = np.zeros((NT, 128, 512), dtype=BF16_NP)
        tail_x = (xc[:, KF * 128:].reshape(NT, 512, KTAIL)
                  .transpose(0, 2, 1))                   # [nt, j, b]
        for r in range(4):
            xt6[:, 32 * r:32 * r + KTAIL, :] = tail_x
        in_maps.append({"xt": xt, "xt6": xt6, "w1": w1_host,
                        "w1k7": w1k7, "w2": w2_host, "sel": sel_host})
    return in_maps


def _gather(results):
    full = np.concatenate([np.asarray(r["out"], dtype=np.float32)
                           for r in results], axis=1)    # [10, 8192]
    return np.ascontiguousarray(full.T)                  # [8192, 10]


def run(x, W1, b1, W2, b2, trace=False, **trace_kwargs):
    import os
    nc = _build()
    in_maps = _prep_inputs(x, W1, b1, W2, b2)
    if not trace:
        # The NTFF profiling hook isn't available in every environment;
        # make sure an ambient BASS_TRACE can't pull us onto that path.
        os.environ["BASS_NEVER_TRACE"] = "1"
    else:
        os.environ.pop("BASS_NEVER_TRACE", None)
    res = bass_utils.run_bass_kernel_spmd(
        nc, in_maps, core_ids=list(range(NCORES)), trace=trace,
        **trace_kwargs)
    return _gather(res.results), res


def kernel(x, W1, b1, W2, b2):
    out, _ = run(x, W1, b1, W2, b2)
    return out
